# revision 35
# baseline (speedup 1.0000x reference)
"""Adaptive mean thresholding (11x11 box, replicate border, C=0.02) on 8
TRN2 NeuronCores. Batch [128,512,512] f32 -> binary-inv threshold map.

Strategy (pure data parallel, 16 images per core):
  - horizontal 11-tap sliding sum: one DVE tensor_tensor_scan per 128-row
    block  (S[t] = S[t-1] + x[t+5] - x[t-6], replicate-padded, output
    mean-centered via the scan's initial value, downcast to fp16)
  - vertical 11-tap sum: TensorE band-matrix matmuls in fp16 (integer
    weights exact in fp16; products exact; PSUM accumulates in f32)
  - epilogue: ACT thresh = psum/121 + 0.48 straight out of PSUM (one op
    per image), DVE is_le(x, thresh) -> {0,1}
  - software-pipelined across the 16 images (epilogue lags the front by
    one image) so every engine queue stays non-blocking; loads on the
    sync HWDGE queue, stores on the gpsimd SWDGE queue (separate rings)
"""

import numpy as np

B, H, W = 128, 512, 512
NCORES = 8
NIMG = B // NCORES          # 16 images per core
P = 128                     # partitions
NB = H // P                 # 4 row blocks per image
K = 11                      # box size
PADL, PADR = 6, 5           # left pad 6 (for x[t-6]), right pad 5 (x[t+5])
XPW = 524                   # padded block width (512 + 11 pads + 1 align)
CONST = 0.02

_CACHE = {}


def _band_weights():
    """Full 512x512 vertical box-filter count matrix, sliced to the five
    distinct 128x128 blocks the kernel needs."""
    Bm = np.zeros((H, H), dtype=np.float32)
    for i in range(H):
        for d in range(-5, 6):
            r = min(max(i + d, 0), H - 1)
            Bm[r, i] += 1.0
    W0 = Bm[0:128, 0:128]        # block 0 main (top replicate folded)
    WI = Bm[128:256, 128:256]    # interior main (pure band)
    W3 = Bm[384:512, 384:512]    # block 3 main (bottom replicate folded)
    WDN = Bm[0:128, 128:256]     # contribution from tile I-1 to block I
    WUP = Bm[128:256, 0:128]     # contribution from tile I+1 to block I
    return np.ascontiguousarray(np.stack([W0, WI, W3, WDN, WUP])).astype(np.float16)


def _build():
    import concourse.bass as bass  # noqa: F401
    import concourse.tile as tile
    from concourse import bacc, mybir
    from concourse.alu_op_type import AluOpType

    F32 = mybir.dt.float32
    F16 = mybir.dt.float16
    ACT_COPY = mybir.ActivationFunctionType.Copy

    nc = bacc.Bacc("TRN2", target_bir_lowering=False, debug=False,
                   num_devices=NCORES)
    in_ext = nc.dram_tensor("input", [NIMG, H, W], F32,
                            kind="ExternalInput").ap()
    wts_ext = nc.dram_tensor("wts", [5, 128, 128], F16,
                             kind="ExternalInput").ap()
    out_ext = nc.dram_tensor("output", [NIMG, H, W], F32,
                             kind="ExternalOutput").ap()

    with tile.TileContext(nc) as tc:
        with tc.tile_pool(name="consts", bufs=1) as consts, \
             tc.tile_pool(name="xp", bufs=5) as xp_pool, \
             tc.tile_pool(name="sh", bufs=4) as sh_pool, \
             tc.tile_pool(name="th", bufs=4) as th_pool, \
             tc.tile_pool(name="ot", bufs=4) as ot_pool, \
             tc.tile_pool(name="small", bufs=4) as small, \
             tc.tile_pool(name="psum", bufs=2, space="PSUM") as psum:

            # --- constants: band weight blocks (fp16 direct from host).
            # The DMA is emitted lazily (first matmul need) so the first
            # image load wins the sync queue at startup.
            wt = consts.tile([P, 5 * 128], F16)
            wv = wt[:].rearrange("p (s m) -> p s m", s=5)
            W0, WI, W3, WDN, WUP = (wv[:, s, :] for s in range(5))
            MAIN = (W0, WI, WI, W3)
            neg_half = consts.tile([P, 1], F32)
            nc.gpsimd.memset(neg_half[:], -0.5)
            wts_loaded = []

            def load_wts():
                if not wts_loaded:
                    nc.sync.dma_start(
                        wt[:].rearrange("p (s m) -> p s m", s=5),
                        wts_ext.rearrange("s p m -> p s m"))
                    wts_loaded.append(True)

            live = {}

            def front(im):
                # load/pad/scan per quarter so early blocks flow sooner
                nhalf = 4
                xp = xp_pool.tile([P, NB * XPW], F32, tag="xp")
                xpv = xp[:].rearrange("p (b c) -> p b c", b=NB)
                src = in_ext[im].rearrange("(b p) w -> p b w", p=P)
                init = small.tile([P, NB], F32, tag="init")
                scr = small.tile([P, NB * K], F32, tag="scr")
                sh = sh_pool.tile([P, NB * W], F16, tag="sh")
                hb = NB // nhalf
                for h in range(nhalf):
                    blo, bhi = h * hb, (h + 1) * hb
                    nc.sync.dma_start(
                        xpv[:, blo:bhi, PADL:PADL + W], src[:, blo:bhi])
                    # replicate-pad left/right edges (ACT engine)
                    nc.scalar.activation(
                        xpv[:, blo:bhi, 0:PADL],
                        xpv[:, blo:bhi, PADL:PADL + 1]
                        .broadcast_to([P, bhi - blo, PADL]),
                        ACT_COPY, bias=0.0, scale=1.0)
                    nc.scalar.activation(
                        xpv[:, blo:bhi, PADL + W:PADL + W + PADR],
                        xpv[:, blo:bhi, PADL + W - 1:PADL + W]
                        .broadcast_to([P, bhi - blo, PADR]),
                        ACT_COPY, bias=0.0, scale=1.0)
                    # scan initial values: sum(xp[b, 0:11] - 0.5) per block
                    for b in range(blo, bhi):
                        nc.scalar.activation(
                            scr[:, K * b:K * (b + 1)], xpv[:, b, 0:K],
                            mybir.ActivationFunctionType.Identity,
                            bias=neg_half[:], scale=1.0,
                            accum_out=init[:, b:b + 1])
                    # horizontal sliding sums (mean-centered, fp16 out)
                    for b in range(blo, bhi):
                        nc.vector.tensor_tensor_scan(
                            out=sh[:, W * b:W * (b + 1)],
                            data0=xp[:, XPW * b + K:XPW * b + K + W],
                            data1=xp[:, XPW * b:XPW * b + W],
                            initial=init[:, b:b + 1],
                            op0=AluOpType.add, op1=AluOpType.subtract)

                # vertical band matmuls (fp16, f32 PSUM accumulate)
                load_wts()
                ps = psum.tile([P, NB * W], F32, tag="ps")
                for b in range(NB):
                    psb = ps[:, W * b:W * (b + 1)]
                    sht = lambda t: sh[:, W * t:W * (t + 1)]  # noqa: E731
                    if b == 0:
                        nc.tensor.matmul(psb, MAIN[0], sht(0),
                                         start=True, stop=False)
                        nc.tensor.matmul(psb, WUP, sht(1),
                                         start=False, stop=True)
                    elif b == NB - 1:
                        nc.tensor.matmul(psb, WDN, sht(b - 1),
                                         start=True, stop=False)
                        nc.tensor.matmul(psb, MAIN[b], sht(b),
                                         start=False, stop=True)
                    else:
                        nc.tensor.matmul(psb, WDN, sht(b - 1),
                                         start=True, stop=False)
                        nc.tensor.matmul(psb, MAIN[b], sht(b),
                                         start=False, stop=False)
                        nc.tensor.matmul(psb, WUP, sht(b + 1),
                                         start=False, stop=True)
                live[im] = (xp, xpv, ps)

            def epilogue(im):
                xp, xpv, ps = live.pop(im)
                # the last image drains per half so stores overlap compares
                nhalf = 4 if im == NIMG - 1 else 1
                th = th_pool.tile([P, NB * W], F32, tag="th")
                ot = ot_pool.tile([P, NB * W], F32, tag="ot")
                thv = th[:].rearrange("p (b c) -> p b c", b=NB)
                otv = ot[:].rearrange("p (b c) -> p b c", b=NB)
                dst = out_ext[im].rearrange("(b p) w -> p b w", p=P)
                hb = NB // nhalf
                for h in range(nhalf):
                    blo, bhi = h * hb, (h + 1) * hb
                    # thresh = (S2d - 60.5)/121 + 0.48 = mean - 0.02
                    nc.scalar.activation(
                        thv[:, blo:bhi], ps[:, W * blo:W * bhi], ACT_COPY,
                        bias=0.5 - CONST, scale=1.0 / (K * K))
                    # out = (x <= thresh) as 1.0/0.0  (DVE)
                    nc.vector.tensor_tensor(
                        out=otv[:, blo:bhi],
                        in0=thv[:, blo:bhi],
                        in1=xpv[:, blo:bhi, PADL:PADL + W],
                        op=AluOpType.is_ge)
                    # store on gpsimd SWDGE; final image via sync so the
                    # tail gpsimd dge-drain isn't waiting on a live transfer
                    eng = nc.sync if im == NIMG - 1 else nc.gpsimd
                    eng.dma_start(dst[:, blo:bhi], otv[:, blo:bhi])

            # software pipeline: epilogue of image i emitted after the
            # front of image i+1, so no engine queue blocks the next image
            for im in range(NIMG):
                front(im)
                if im >= 1:
                    epilogue(im - 1)
            epilogue(NIMG - 1)

    nc.compile()
    return nc


def _get_nc():
    if "nc" not in _CACHE:
        _CACHE["nc"] = _build()
        _CACHE["wts"] = _band_weights()
    return _CACHE["nc"]


def kernel(input_batch: np.ndarray) -> np.ndarray:
    from concourse.bass_utils import run_bass_kernel_spmd

    nc = _get_nc()
    wts = _CACHE["wts"]
    assert input_batch.shape == (B, H, W)
    x = np.ascontiguousarray(input_batch, dtype=np.float32)
    in_maps = [
        {"input": x[c * NIMG:(c + 1) * NIMG], "wts": wts}
        for c in range(NCORES)
    ]
    res = run_bass_kernel_spmd(nc, in_maps, core_ids=list(range(NCORES)))
    return np.concatenate([r["output"] for r in res.results], axis=0)


if __name__ == "__main__":
    rng = np.random.default_rng(0)
    x = rng.random((B, H, W), dtype=np.float32)
    y = kernel(x)
    print(y.shape, y.dtype, y.mean())
